# revision 27
# baseline (speedup 1.0000x reference)
"""NetVLAD Trainium2 kernel v6 — host-shipped transpose, PE-computed ssq,
flat 2D access patterns, packed constants, single-queue input staging.

Host ships x twice per image: xcp [C, P] fp16 (u-matmul stationary) and
xts [128px, 32t, 132] fp16 (vlad rhs; col 128 filled on-device with |x_p|),
plus one packed const tensor (cf f32 | cwo f16 | eb8 bf16 | cbi bf16).

Per 1024-px chunk (8 px-tiles), stages pipelined across slots:
  A: ACT sq2 = x^2 ([C,1024]); PE 8 u-matmuls (psU[128,8,64]) + 8 ssq
     matmuls (psS[:,j] = sq2_j^T @ ones); ACT lssq=ln(psS),
     invc=exp(-.5 lssq), ncol=exp(+.5 lssq) -> xts[:,:,128]
  B: DVE ls = psU*invcB (f16); DVE negm reduce; GPS d = ls+negmB;
     ACT E = exp(d + theta) -> bf16
  C: GPS EB = E*eb8; DVE scol reduce; DVE gcol=1/scol; DVE rcol;
     GPS Ep = EB[:, :, 0:56]*rcolB
  V (lag 3): PE psV[56,0:129] += Ep_j.T @ xts[:,j,0:129]
"""

import sys

for _p in ("/opt/trn_rl_repo",):
    if _p not in sys.path:
        sys.path.insert(0, _p)

import numpy as np

NIMG = 4
C = 128
K = 64
KE = 56
P = 4096
NCH = 4        # 1024-px chunks per image
TPC = 8        # 128-px tiles per chunk
NSLOT = NIMG * NCH
LAG = 3
THETA = 35.0

# packed const layout (bytes per partition):
#   [0:1028)    cf  f32 [257]: cen rows0:56 cols0:128 | theta col 128 |
#               onesrow row0 cols 129:257
#   [1028:1158) cwo f16 [65]:  w^T cols 0:64 | ones col 64
#   [1158:2182) eb8 bf16 [512]: exp(b - bmid - theta) tiled x8
#   [2182:2438) cbi bf16 [128]: identity
CPACK = 2440

_cache = {}


def _build():
    import concourse.mybir as mybir
    from concourse import bacc, tile
    from concourse.hw_specs import get_activation_tables

    f32 = mybir.dt.float32
    f16 = mybir.dt.float16
    bf16 = mybir.dt.bfloat16
    u8 = mybir.dt.uint8
    Alu = mybir.AluOpType
    Act = mybir.ActivationFunctionType

    nc = bacc.Bacc()
    xcp_in = nc.declare_dram_parameter("xcp", [NIMG, C, P], f16, isOutput=False)
    xts_in = nc.declare_dram_parameter("xts", [NIMG, C, 32, 132], f16,
                                       isOutput=False)
    cp_in = nc.declare_dram_parameter("cpack", [C, CPACK], u8, isOutput=False)
    out_ext = nc.declare_dram_parameter("out", [NIMG, KE, C], f32,
                                        isOutput=True)

    tabs = list(get_activation_tables(nc.m.arch).keys())
    tab_id = tabs.index("natural_log_exp_and_others")

    with tile.TileContext(nc) as tc:
        with (
            tc.tile_pool(name="const", bufs=1) as cpool,
            tc.tile_pool(name="xq", bufs=4) as xqpool,
            tc.tile_pool(name="xb", bufs=3) as xbpool,
            tc.tile_pool(name="xts", bufs=4) as tpool,
            tc.tile_pool(name="sq", bufs=3) as qpool,
            tc.tile_pool(name="ls", bufs=3) as lpool,
            tc.tile_pool(name="ew", bufs=3) as epool,
            tc.tile_pool(name="stats", bufs=6) as spool,
            tc.tile_pool(name="fin", bufs=2) as fpool,
            tc.tile_pool(name="psU", bufs=3, space="PSUM") as pU,
            tc.tile_pool(name="psS", bufs=2, space="PSUM") as pS,
            tc.tile_pool(name="psV", bufs=2, space="PSUM") as pV,
            tc.tile_pool(name="psT", bufs=1, space="PSUM") as pT,
        ):
            nc.scalar.add_instruction(mybir.InstLoadActFuncSet(
                name=nc.get_next_instruction_name(), ins=[], outs=[],
                act_func_set_id=tab_id))

            cpk = cpool.tile([C, CPACK], u8, tag="cpk")
            nc.sync.dma_start(cpk[:], cp_in[:])
            cf32 = cpk[:, 0:1028].bitcast(f32)
            cwo = cpk[:, 1028:1158].bitcast(f16)
            eb8 = cpk[:, 1158:2182].bitcast(bf16)
            cbi = cpk[:, 2182:2438].bitcast(bf16)

            cen = cf32[0:KE, 0:C]
            thetacol = cf32[:, 128:129]
            onesrow = cf32[0:1, 129:257]
            ident_b = cbi[:, 0:C]
            ident56_b = cbi[0:KE, 0:KE]
            wslice = cwo[:, 0:K]
            onecol = cwo[:, K:K + 1]

            # x [c,p] for u-matmul stationary; img0 in halves on sync ring,
            # xts (transposed) on gpsimd ring
            xq = []
            xts = {}
            xts[0] = tpool.tile([C, 32, 132], f16, tag="xts", name="xts0")
            for p4 in range(4):
                t = xqpool.tile([C, P // 4], f16, tag="xq", name="xqt")
                nc.sync.dma_start(t[:], xcp_in[0, :, p4 * 1024:(p4 + 1) * 1024])
                xq.append(t)
                nc.gpsimd.dma_start(xts[0][:, p4 * 8:(p4 + 1) * 8, :],
                                    xts_in[0, :, p4 * 8:(p4 + 1) * 8, :])
            xb = {}

            def fetch_xb(img):
                xb[img] = xbpool.tile([C, P], f16, tag="xb", name="xbt")
                nc.sync.dma_start(xb[img][:], xcp_in[img])

            def fetch_xts(img):
                xts[img] = tpool.tile([C, 32, 132], f16, tag="xts",
                                      name="xtst")
                nc.gpsimd.dma_start(xts[img][:], xts_in[img])

            def xchunk(c):
                img, ch = divmod(c, NCH)
                if img == 0:
                    return xq[ch][:]
                return xb[img][:, ch * 1024:(ch + 1) * 1024]

            def xsrc(c, j):
                return xchunk(c)[:, j * 128:(j + 1) * 128]

            def xv(c):
                img, ch = divmod(c, NCH)
                return xts[img][:, ch * TPC:(ch + 1) * TPC, :]

            psU = {}
            psS = {}
            st = {}
            et = {}
            ebt = {}
            ept = {}
            psV = {}
            vk = {}
            tailseq = []

            def stage_a(c):
                img, ch = divmod(c, NCH)
                if img + 1 < NIMG:
                    if ch == 0 and img + 1 not in xb:
                        fetch_xb(img + 1)
                    if ch == 1 and img + 1 not in xts:
                        fetch_xts(img + 1)
                sq2 = qpool.tile([C, 1024], f16, tag="sq2", name="sq2")
                nc.scalar.activation(sq2[:], xchunk(c), Act.Square)
                psU[c] = pU.tile([C, TPC, K], f32, tag="psU", name="psUt")
                for j in range(TPC):
                    nc.tensor.matmul(psU[c][:, j:j + 1, :], xsrc(c, j),
                                     wslice, start=True, stop=True)
                psS[c] = pS.tile([C, TPC], f32, tag="psS", name="psSt")
                for j in range(TPC):
                    nc.tensor.matmul(psS[c][:, j:j + 1],
                                     sq2[:, j * 128:(j + 1) * 128],
                                     onecol, start=True, stop=True)
                s = {}
                s["lssq"] = spool.tile([C, TPC], f32, tag="lssq", name="lssq")
                nc.scalar.activation(s["lssq"][:], psS[c][:], Act.Ln)
                s["invc"] = spool.tile([C, TPC], f32, tag="invc", name="invc")
                nc.scalar.activation(s["invc"][:], s["lssq"][:], Act.Exp,
                                     scale=-0.5)
                ncol = xv(c)[:, :, 128:129].rearrange("p t o -> p (t o)")
                nc.scalar.activation(ncol, s["lssq"][:], Act.Exp, scale=0.5)
                st[c] = s

            def stage_b(c):
                s = st[c]
                ls = lpool.tile([C, TPC * K], f16, tag="ls", name="ls")
                nc.vector.tensor_tensor(
                    ls[:].rearrange("p (t k) -> p t k", k=K), psU[c][:, :, :],
                    s["invc"][:].broadcast_to([C, TPC, K]), Alu.mult)
                s["negm"] = spool.tile([C, TPC], f32, tag="negm", name="negm")
                nc.vector.tensor_reduce(
                    s["negm"][:], ls[:].rearrange("p (t k) -> p t k", k=K),
                    axis=mybir.AxisListType.X, op=Alu.max, negate=True)
                d = lpool.tile([C, TPC * K], f16, tag="d", name="dt")
                nc.gpsimd.tensor_tensor(
                    d[:].rearrange("p (t k) -> p t k", k=K),
                    ls[:].rearrange("p (t k) -> p t k", k=K),
                    s["negm"][:].broadcast_to([C, TPC, K]), Alu.add)
                et[c] = epool.tile([C, TPC * K], bf16, tag="E", name="et")
                nc.scalar.activation(et[c][:], d[:], Act.Exp, bias=thetacol)

            def stage_c1(c):
                ebt[c] = epool.tile([C, TPC * K], bf16, tag="EB", name="ebt")
                nc.gpsimd.tensor_tensor(ebt[c][:], et[c][:], eb8, Alu.mult)

            def stage_c2(c):
                s = st[c]
                s["scol"] = spool.tile([C, TPC], f32, tag="scol", name="scol")
                nc.vector.tensor_reduce(
                    s["scol"][:], ebt[c][:].rearrange("p (t k) -> p t k", k=K),
                    axis=mybir.AxisListType.X, op=Alu.add)
                s["gcol"] = spool.tile([C, TPC], f32, tag="gcol", name="gcol")
                nc.vector.reciprocal(s["gcol"][:], s["scol"][:])
                s["rcol"] = spool.tile([C, TPC], f32, tag="rcol", name="rcol")
                nc.vector.tensor_tensor(s["rcol"][:], s["invc"][:],
                                        s["gcol"][:], Alu.mult)
                ept[c] = epool.tile([C, TPC, KE], bf16, tag="Ep", name="ept")
                nc.gpsimd.tensor_tensor(
                    ept[c][:, :, :],
                    ebt[c][:].rearrange("p (t k) -> p t k", k=K)[:, :, 0:KE],
                    s["rcol"][:].broadcast_to([C, TPC, KE]), Alu.mult)

            def vlads(c):
                img, ch = divmod(c, NCH)
                if ch == 0:
                    psV[img] = pV.tile([KE, 132], f32, tag="psV", name="psVt")
                for j in range(TPC):
                    nc.tensor.matmul(psV[img][0:KE, 0:129],
                                     ept[c][:, j, :],
                                     xv(c)[:, j, 0:129],
                                     start=(ch == 0 and j == 0),
                                     stop=(ch == NCH - 1 and j == TPC - 1))

            def tail_a(img):
                pv = psV[img]
                negs = spool.tile([KE, 1], f32, tag="negs")
                nc.vector.tensor_scalar_mul(negs[:], pv[0:KE, 128:129], -1.0)
                vk[img] = fpool.tile([KE, C], bf16, tag="vk", name="vkt")
                nc.vector.scalar_tensor_tensor(vk[img][:], cen, negs[:],
                                               pv[0:KE, 0:C],
                                               Alu.mult, Alu.add)

            def tail_b(img):
                tt = pT.tile([C, 2, 192], f32, tag="pst", name="pst")
                t1 = tt[:, 0:1, 0:KE]
                nc.tensor.matmul(t1, vk[img][:], ident56_b,
                                 start=True, stop=True)
                tr56 = spool.tile([C, KE], bf16, tag="tr56")
                ssqk = spool.tile([C, 1], f32, tag="ssqk")
                nc.scalar.activation(tr56[:], t1, Act.Square,
                                     accum_out=ssqk[:])
                ssqc = spool.tile([C, 1], f32, tag="ssqc")
                nc.vector.tensor_scalar_max(ssqc[:], ssqk[:], 1e-24)
                lk = spool.tile([C, 1], f32, tag="lk")
                nc.scalar.activation(lk[:], ssqc[:], Act.Ln)
                invk = spool.tile([C, 1], f32, tag="invk")
                nc.scalar.activation(invk[:], lk[:], Act.Exp, scale=-0.5)
                t2 = spool.tile([C, 1], f32, tag="t2")
                nc.vector.scalar_tensor_tensor(t2[:], ssqc[:], invk[:],
                                               invk[:], Alu.mult, Alu.mult)
                tot = spool.tile([1, 1], f32, tag="tot")
                nc.gpsimd.tensor_reduce(tot[:], t2[:],
                                        axis=mybir.AxisListType.C, op=Alu.add)
                totc = spool.tile([1, 1], f32, tag="totc")
                nc.vector.tensor_scalar_max(totc[:], tot[:], 1e-24)
                ltot = spool.tile([1, 1], f32, tag="ltot")
                nc.scalar.activation(ltot[:], totc[:], Act.Ln)
                fv = spool.tile([1, 1], f32, tag="fv")
                nc.scalar.activation(fv[:], ltot[:], Act.Exp, scale=-0.5)
                nc.tensor.matmul(tt[:, 1:2, 188:189], onesrow, fv[:],
                                 start=True, stop=True)
                comb = spool.tile([C, 1], f32, tag="comb")
                nc.vector.tensor_tensor(comb[:], invk[:],
                                        tt[:, 1:2, 188:189], Alu.mult)
                vnT = fpool.tile([C, KE], bf16, tag="vnT", name="vnT")
                nc.vector.tensor_scalar(vnT[:], t1, comb[:], None, Alu.mult)
                return tt, vnT

            def tail_c(img, tt, vnT):
                nc.tensor.matmul(tt[0:KE, 1:2, 0:C], vnT[:], ident_b,
                                 start=True, stop=True)
                ob = fpool.tile([KE, C], f32, tag="ob", name="ob")
                nc.scalar.activation(ob[:], tt[0:KE, 1:2, 0:C], Act.Copy)
                nc.sync.dma_start(out_ext[img], ob[:])

            for sl in range(NSLOT + LAG + 2):
                if 0 <= sl - 2 < NSLOT:
                    stage_c1(sl - 2)
                if sl < NSLOT:
                    stage_a(sl)
                if 0 <= sl - 1 < NSLOT:
                    stage_b(sl - 1)
                if 0 <= sl - 2 < NSLOT:
                    stage_c2(sl - 2)
                v = sl - LAG
                if 0 <= v < NSLOT:
                    vlads(v)
                    img, ch = divmod(v, NCH)
                    if ch == NCH - 1:
                        tail_a(img)

                        def _mk(i):
                            def _b():
                                tt, vnT = tail_b(i)
                                tailseq.append(
                                    (sl + 3, lambda: tail_c(i, tt, vnT)))
                            return _b
                        tailseq.append((sl + 1, _mk(img)))
                while tailseq and tailseq[0][0] <= sl:
                    tailseq.pop(0)[1]()
            while tailseq:
                tailseq.pop(0)[1]()

    nc.compile()
    return nc


def _get_nc():
    if "nc" not in _cache:
        _cache["nc"] = _build()
    return _cache["nc"]


def _make_in_maps(x, conv_w, conv_b, centroids):
    import ml_dtypes

    x = np.asarray(x, dtype=np.float32)
    conv_w = np.asarray(conv_w, dtype=np.float32)
    conv_b = np.asarray(conv_b, dtype=np.float32)
    centroids = np.asarray(centroids, dtype=np.float32)

    N = x.shape[0]
    n_cores = 8
    per = N // n_cores
    assert per == NIMG

    xr = x.reshape(N, C, P).astype(np.float16)
    bmid = (conv_b.max() + conv_b.min()) / 2.0
    eb = np.exp((conv_b - bmid - THETA).astype(np.float64)).astype(np.float32)

    cf = np.zeros((C, 257), dtype=np.float32)
    cf[0:KE, 0:C] = centroids[:KE]
    cf[:, 128] = THETA
    cf[0, 129:257] = 1.0
    cwo = np.zeros((C, K + 1), dtype=np.float16)
    cwo[:, 0:K] = conv_w.T.astype(np.float16)
    cwo[:, K] = 1.0
    eb8 = np.broadcast_to(np.tile(eb, TPC)[None, :], (C, TPC * K)).astype(
        ml_dtypes.bfloat16)
    cbi = np.eye(C, dtype=np.float32).astype(ml_dtypes.bfloat16)

    cpack = np.concatenate([
        np.ascontiguousarray(cf).view(np.uint8),
        np.ascontiguousarray(cwo).view(np.uint8),
        np.ascontiguousarray(eb8).view(np.uint8),
        np.ascontiguousarray(cbi).view(np.uint8),
        np.zeros((C, 2), dtype=np.uint8),
    ], axis=1)
    assert cpack.shape == (C, CPACK), cpack.shape

    in_maps = []
    for i in range(n_cores):
        xc = np.ascontiguousarray(xr[i * per:(i + 1) * per])
        # xts[img, q, t, c] = x[img, c, t*128+q], padded to 132 cols
        xt = np.zeros((NIMG, C, 32, 132), dtype=np.float16)
        xt[:, :, :, 0:C] = xc.reshape(NIMG, C, 32, C).transpose(0, 3, 2, 1)
        in_maps.append({
            "xcp": xc,
            "xts": np.ascontiguousarray(xt),
            "cpack": cpack,
        })
    return in_maps


def kernel(x, conv_w, conv_b, centroids):
    from concourse.bass_utils import run_bass_kernel_spmd

    in_maps = _make_in_maps(x, conv_w, conv_b, centroids)
    nc = _get_nc()
    res = run_bass_kernel_spmd(nc, in_maps, list(range(8)))
    outs = [np.asarray(r["out"]).reshape(NIMG, KE * C) for r in res.results]
    return np.concatenate(outs, axis=0)


if __name__ == "__main__":
    rng = np.random.default_rng(0)
    x = rng.standard_normal((32, C, 64, 64), dtype=np.float32)
    w = rng.standard_normal((K, C), dtype=np.float32)
    b = rng.standard_normal((K,), dtype=np.float32)
    c = rng.random((K, C), dtype=np.float32)
    out = kernel(x=x, conv_w=w, conv_b=b, centroids=c)
    print(out.shape, out.dtype)


# revision 28
# speedup vs baseline: 1.0273x; 1.0273x over previous
"""NetVLAD Trainium2 kernel v6 — host-shipped transpose, PE-computed ssq,
flat 2D access patterns, packed constants, single-queue input staging.

Host ships x twice per image: xcp [C, P] fp16 (u-matmul stationary) and
xts [128px, 32t, 132] fp16 (vlad rhs; col 128 filled on-device with |x_p|),
plus one packed const tensor (cf f32 | cwo f16 | eb8 bf16 | cbi bf16).

Per 1024-px chunk (8 px-tiles), stages pipelined across slots:
  A: ACT sq2 = x^2 ([C,1024]); PE 8 u-matmuls (psU[128,8,64]) + 8 ssq
     matmuls (psS[:,j] = sq2_j^T @ ones); ACT lssq=ln(psS),
     invc=exp(-.5 lssq), ncol=exp(+.5 lssq) -> xts[:,:,128]
  B: DVE ls = psU*invcB (f16); DVE negm reduce; GPS d = ls+negmB;
     ACT E = exp(d + theta) -> bf16
  C: GPS EB = E*eb8; DVE scol reduce; DVE gcol=1/scol; DVE rcol;
     GPS Ep = EB[:, :, 0:56]*rcolB
  V (lag 3): PE psV[56,0:129] += Ep_j.T @ xts[:,j,0:129]
"""

import sys

for _p in ("/opt/trn_rl_repo",):
    if _p not in sys.path:
        sys.path.insert(0, _p)

import numpy as np

NIMG = 4
C = 128
K = 64
KE = 56
P = 4096
NCH = 4        # 1024-px chunks per image
TPC = 8        # 128-px tiles per chunk
NSLOT = NIMG * NCH
LAG = 3
THETA = 35.0

# packed const layout (bytes per partition):
#   [0:1028)    cf  f32 [257]: cen rows0:56 cols0:128 | theta col 128 |
#               onesrow row0 cols 129:257
#   [1028:1158) cwo f16 [65]:  w^T cols 0:64 | ones col 64
#   [1158:2182) eb8 bf16 [512]: exp(b - bmid - theta) tiled x8
#   [2182:2438) cbi bf16 [128]: identity
CPACK = 2440

_cache = {}


def _build():
    import concourse.mybir as mybir
    from concourse import bacc, tile
    from concourse.hw_specs import get_activation_tables

    f32 = mybir.dt.float32
    f16 = mybir.dt.float16
    bf16 = mybir.dt.bfloat16
    u8 = mybir.dt.uint8
    Alu = mybir.AluOpType
    Act = mybir.ActivationFunctionType

    nc = bacc.Bacc()
    xcp_in = nc.declare_dram_parameter("xcp", [NIMG, C, P], f16, isOutput=False)
    xts_in = nc.declare_dram_parameter("xts", [NIMG, C, 32, 132], f16,
                                       isOutput=False)
    cp_in = nc.declare_dram_parameter("cpack", [C, CPACK], u8, isOutput=False)
    out_ext = nc.declare_dram_parameter("out", [NIMG, KE, C], f32,
                                        isOutput=True)

    tabs = list(get_activation_tables(nc.m.arch).keys())
    tab_id = tabs.index("natural_log_exp_and_others")

    with tile.TileContext(nc) as tc:
        with (
            tc.tile_pool(name="const", bufs=1) as cpool,
            tc.tile_pool(name="xq", bufs=4) as xqpool,
            tc.tile_pool(name="xb", bufs=3) as xbpool,
            tc.tile_pool(name="xts", bufs=4) as tpool,
            tc.tile_pool(name="sq", bufs=2) as qpool,
            tc.tile_pool(name="ls", bufs=2) as lpool,
            tc.tile_pool(name="ew", bufs=3) as epool,
            tc.tile_pool(name="stats", bufs=4) as spool,
            tc.tile_pool(name="fin", bufs=2) as fpool,
            tc.tile_pool(name="psU", bufs=2, space="PSUM") as pU,
            tc.tile_pool(name="psS", bufs=2, space="PSUM") as pS,
            tc.tile_pool(name="psV", bufs=2, space="PSUM") as pV,
            tc.tile_pool(name="psT", bufs=2, space="PSUM") as pT,
        ):
            nc.scalar.add_instruction(mybir.InstLoadActFuncSet(
                name=nc.get_next_instruction_name(), ins=[], outs=[],
                act_func_set_id=tab_id))

            cpk = cpool.tile([C, CPACK], u8, tag="cpk")
            nc.sync.dma_start(cpk[:], cp_in[:])
            cf32 = cpk[:, 0:1028].bitcast(f32)
            cwo = cpk[:, 1028:1158].bitcast(f16)
            eb8 = cpk[:, 1158:2182].bitcast(bf16)
            cbi = cpk[:, 2182:2438].bitcast(bf16)

            cen = cf32[0:KE, 0:C]
            thetacol = cf32[:, 128:129]
            onesrow = cf32[0:1, 129:257]
            ident_b = cbi[:, 0:C]
            ident56_b = cbi[0:KE, 0:KE]
            wslice = cwo[:, 0:K]
            onecol = cwo[:, K:K + 1]

            # x [c,p] for u-matmul stationary; img0 in halves on sync ring,
            # xts (transposed) on gpsimd ring
            xq = []
            xts = {}
            xts[0] = tpool.tile([C, 32, 132], f16, tag="xts", name="xts0")
            for h in range(2):
                t = xqpool.tile([C, P // 2], f16, tag="xq", name="xqt")
                nc.sync.dma_start(t[:], xcp_in[0, :, h * 2048:(h + 1) * 2048])
                xq.append(t)
                nc.gpsimd.dma_start(xts[0][:, h * 16:(h + 1) * 16, :],
                                    xts_in[0, :, h * 16:(h + 1) * 16, :])
            xb = {}
            xb[1] = xbpool.tile([C, P], f16, tag="xb", name="xbt")
            nc.sync.dma_start(xb[1][:], xcp_in[1])
            xts[1] = tpool.tile([C, 32, 132], f16, tag="xts", name="xts1")
            nc.gpsimd.dma_start(xts[1][:], xts_in[1])

            def xchunk(c):
                img, ch = divmod(c, NCH)
                if img == 0:
                    return xq[ch // 2][:, (ch % 2) * 1024:(ch % 2 + 1) * 1024]
                return xb[img][:, ch * 1024:(ch + 1) * 1024]

            def xsrc(c, j):
                return xchunk(c)[:, j * 128:(j + 1) * 128]

            def xv(c):
                img, ch = divmod(c, NCH)
                return xts[img][:, ch * TPC:(ch + 1) * TPC, :]

            psU = {}
            psS = {}
            st = {}
            et = {}
            ebt = {}
            ept = {}
            psV = {}
            vk = {}
            tailseq = []

            def stage_a(c):
                img, ch = divmod(c, NCH)
                if ch == 0 and img + 1 < NIMG and img + 1 not in xb:
                    xb[img + 1] = xbpool.tile([C, P], f16, tag="xb",
                                              name="xbt")
                    nc.sync.dma_start(xb[img + 1][:], xcp_in[img + 1])
                    xts[img + 1] = tpool.tile([C, 32, 132], f16, tag="xts",
                                              name="xtst")
                    nc.gpsimd.dma_start(xts[img + 1][:], xts_in[img + 1])
                sq2 = qpool.tile([C, 1024], f16, tag="sq2", name="sq2")
                nc.scalar.activation(sq2[:], xchunk(c), Act.Square)
                psU[c] = pU.tile([C, TPC, K], f32, tag="psU", name="psUt")
                for j in range(TPC):
                    nc.tensor.matmul(psU[c][:, j:j + 1, :], xsrc(c, j),
                                     wslice, start=True, stop=True)
                psS[c] = pS.tile([C, TPC], f32, tag="psS", name="psSt")
                for j in range(TPC):
                    nc.tensor.matmul(psS[c][:, j:j + 1],
                                     sq2[:, j * 128:(j + 1) * 128],
                                     onecol, start=True, stop=True)
                s = {}
                s["lssq"] = spool.tile([C, TPC], f32, tag="lssq", name="lssq")
                nc.scalar.activation(s["lssq"][:], psS[c][:], Act.Ln)
                s["invc"] = spool.tile([C, TPC], f32, tag="invc", name="invc")
                nc.scalar.activation(s["invc"][:], s["lssq"][:], Act.Exp,
                                     scale=-0.5)
                ncol = xv(c)[:, :, 128:129].rearrange("p t o -> p (t o)")
                nc.scalar.activation(ncol, s["lssq"][:], Act.Exp, scale=0.5)
                st[c] = s

            def stage_b(c):
                s = st[c]
                ls = lpool.tile([C, TPC * K], f16, tag="ls", name="ls")
                nc.vector.tensor_tensor(
                    ls[:].rearrange("p (t k) -> p t k", k=K), psU[c][:, :, :],
                    s["invc"][:].broadcast_to([C, TPC, K]), Alu.mult)
                s["negm"] = spool.tile([C, TPC], f32, tag="negm", name="negm")
                nc.vector.tensor_reduce(
                    s["negm"][:], ls[:].rearrange("p (t k) -> p t k", k=K),
                    axis=mybir.AxisListType.X, op=Alu.max, negate=True)
                d = lpool.tile([C, TPC * K], f16, tag="d", name="dt")
                nc.gpsimd.tensor_tensor(
                    d[:].rearrange("p (t k) -> p t k", k=K),
                    ls[:].rearrange("p (t k) -> p t k", k=K),
                    s["negm"][:].broadcast_to([C, TPC, K]), Alu.add)
                et[c] = epool.tile([C, TPC * K], bf16, tag="E", name="et")
                nc.scalar.activation(et[c][:], d[:], Act.Exp, bias=thetacol)

            def stage_c(c):
                s = st[c]
                ebt[c] = epool.tile([C, TPC * K], bf16, tag="EB", name="ebt")
                nc.gpsimd.tensor_tensor(ebt[c][:], et[c][:], eb8, Alu.mult)
                s["scol"] = spool.tile([C, TPC], f32, tag="scol", name="scol")
                nc.vector.tensor_reduce(
                    s["scol"][:], ebt[c][:].rearrange("p (t k) -> p t k", k=K),
                    axis=mybir.AxisListType.X, op=Alu.add)
                s["gcol"] = spool.tile([C, TPC], f32, tag="gcol", name="gcol")
                nc.vector.reciprocal(s["gcol"][:], s["scol"][:])
                s["rcol"] = spool.tile([C, TPC], f32, tag="rcol", name="rcol")
                nc.vector.tensor_tensor(s["rcol"][:], s["invc"][:],
                                        s["gcol"][:], Alu.mult)
                ept[c] = epool.tile([C, TPC, KE], bf16, tag="Ep", name="ept")
                nc.gpsimd.tensor_tensor(
                    ept[c][:, :, :],
                    ebt[c][:].rearrange("p (t k) -> p t k", k=K)[:, :, 0:KE],
                    s["rcol"][:].broadcast_to([C, TPC, KE]), Alu.mult)

            def vlads(c):
                img, ch = divmod(c, NCH)
                if ch == 0:
                    psV[img] = pV.tile([KE, 132], f32, tag="psV", name="psVt")
                for j in range(TPC):
                    nc.tensor.matmul(psV[img][0:KE, 0:129],
                                     ept[c][:, j, :],
                                     xv(c)[:, j, 0:129],
                                     start=(ch == 0 and j == 0),
                                     stop=(ch == NCH - 1 and j == TPC - 1))

            def tail_a(img):
                pv = psV[img]
                negs = spool.tile([KE, 1], f32, tag="negs")
                nc.vector.tensor_scalar_mul(negs[:], pv[0:KE, 128:129], -1.0)
                vk[img] = fpool.tile([KE, C], bf16, tag="vk", name="vkt")
                nc.vector.scalar_tensor_tensor(vk[img][:], cen, negs[:],
                                               pv[0:KE, 0:C],
                                               Alu.mult, Alu.add)

            def tail_b(img):
                tt = pT.tile([C, 2, 192], f32, tag="pst", name="pst")
                t1 = tt[:, 0:1, 0:KE]
                nc.tensor.matmul(t1, vk[img][:], ident56_b,
                                 start=True, stop=True)
                tr56 = spool.tile([C, KE], bf16, tag="tr56")
                ssqk = spool.tile([C, 1], f32, tag="ssqk")
                nc.scalar.activation(tr56[:], t1, Act.Square,
                                     accum_out=ssqk[:])
                ssqc = spool.tile([C, 1], f32, tag="ssqc")
                nc.vector.tensor_scalar_max(ssqc[:], ssqk[:], 1e-24)
                lk = spool.tile([C, 1], f32, tag="lk")
                nc.scalar.activation(lk[:], ssqc[:], Act.Ln)
                invk = spool.tile([C, 1], f32, tag="invk")
                nc.scalar.activation(invk[:], lk[:], Act.Exp, scale=-0.5)
                t2 = spool.tile([C, 1], f32, tag="t2")
                nc.vector.scalar_tensor_tensor(t2[:], ssqc[:], invk[:],
                                               invk[:], Alu.mult, Alu.mult)
                tot = spool.tile([1, 1], f32, tag="tot")
                nc.gpsimd.tensor_reduce(tot[:], t2[:],
                                        axis=mybir.AxisListType.C, op=Alu.add)
                totc = spool.tile([1, 1], f32, tag="totc")
                nc.vector.tensor_scalar_max(totc[:], tot[:], 1e-24)
                ltot = spool.tile([1, 1], f32, tag="ltot")
                nc.scalar.activation(ltot[:], totc[:], Act.Ln)
                fv = spool.tile([1, 1], f32, tag="fv")
                nc.scalar.activation(fv[:], ltot[:], Act.Exp, scale=-0.5)
                nc.tensor.matmul(tt[:, 1:2, 188:189], onesrow, fv[:],
                                 start=True, stop=True)
                comb = spool.tile([C, 1], f32, tag="comb")
                nc.vector.tensor_tensor(comb[:], invk[:],
                                        tt[:, 1:2, 188:189], Alu.mult)
                vnT = fpool.tile([C, KE], bf16, tag="vnT", name="vnT")
                nc.vector.tensor_scalar(vnT[:], t1, comb[:], None, Alu.mult)
                return tt, vnT

            def tail_c(img, tt, vnT):
                nc.tensor.matmul(tt[0:KE, 1:2, 0:C], vnT[:], ident_b,
                                 start=True, stop=True)
                ob = fpool.tile([KE, C], f32, tag="ob", name="ob")
                nc.scalar.activation(ob[:], tt[0:KE, 1:2, 0:C], Act.Copy)
                nc.sync.dma_start(out_ext[img], ob[:])

            for sl in range(NSLOT + LAG + 2):
                while tailseq and tailseq[0][0] <= sl:
                    tailseq.pop(0)[1]()
                if sl < NSLOT:
                    stage_a(sl)
                v = sl - LAG
                if 0 <= v < NSLOT:
                    vlads(v)
                    img, ch = divmod(v, NCH)
                    if ch == NCH - 1:
                        tail_a(img)

                        def _mk(i):
                            def _b():
                                tt, vnT = tail_b(i)
                                tailseq.append(
                                    (sl + 3, lambda: tail_c(i, tt, vnT)))
                            return _b
                        tailseq.append((sl + 1, _mk(img)))
                if 0 <= sl - 1 < NSLOT:
                    stage_b(sl - 1)
                if 0 <= sl - 2 < NSLOT:
                    stage_c(sl - 2)
            while tailseq:
                tailseq.pop(0)[1]()

    nc.compile()
    return nc


def _get_nc():
    if "nc" not in _cache:
        _cache["nc"] = _build()
    return _cache["nc"]


def _make_in_maps(x, conv_w, conv_b, centroids):
    import ml_dtypes

    x = np.asarray(x, dtype=np.float32)
    conv_w = np.asarray(conv_w, dtype=np.float32)
    conv_b = np.asarray(conv_b, dtype=np.float32)
    centroids = np.asarray(centroids, dtype=np.float32)

    N = x.shape[0]
    n_cores = 8
    per = N // n_cores
    assert per == NIMG

    xr = x.reshape(N, C, P).astype(np.float16)
    bmid = (conv_b.max() + conv_b.min()) / 2.0
    eb = np.exp((conv_b - bmid - THETA).astype(np.float64)).astype(np.float32)

    cf = np.zeros((C, 257), dtype=np.float32)
    cf[0:KE, 0:C] = centroids[:KE]
    cf[:, 128] = THETA
    cf[0, 129:257] = 1.0
    cwo = np.zeros((C, K + 1), dtype=np.float16)
    cwo[:, 0:K] = conv_w.T.astype(np.float16)
    cwo[:, K] = 1.0
    eb8 = np.broadcast_to(np.tile(eb, TPC)[None, :], (C, TPC * K)).astype(
        ml_dtypes.bfloat16)
    cbi = np.eye(C, dtype=np.float32).astype(ml_dtypes.bfloat16)

    cpack = np.concatenate([
        np.ascontiguousarray(cf).view(np.uint8),
        np.ascontiguousarray(cwo).view(np.uint8),
        np.ascontiguousarray(eb8).view(np.uint8),
        np.ascontiguousarray(cbi).view(np.uint8),
        np.zeros((C, 2), dtype=np.uint8),
    ], axis=1)
    assert cpack.shape == (C, CPACK), cpack.shape

    in_maps = []
    for i in range(n_cores):
        xc = np.ascontiguousarray(xr[i * per:(i + 1) * per])
        # xts[img, q, t, c] = x[img, c, t*128+q], padded to 132 cols
        xt = np.zeros((NIMG, C, 32, 132), dtype=np.float16)
        xt[:, :, :, 0:C] = xc.reshape(NIMG, C, 32, C).transpose(0, 3, 2, 1)
        in_maps.append({
            "xcp": xc,
            "xts": np.ascontiguousarray(xt),
            "cpack": cpack,
        })
    return in_maps


def kernel(x, conv_w, conv_b, centroids):
    from concourse.bass_utils import run_bass_kernel_spmd

    in_maps = _make_in_maps(x, conv_w, conv_b, centroids)
    nc = _get_nc()
    res = run_bass_kernel_spmd(nc, in_maps, list(range(8)))
    outs = [np.asarray(r["out"]).reshape(NIMG, KE * C) for r in res.results]
    return np.concatenate(outs, axis=0)


if __name__ == "__main__":
    rng = np.random.default_rng(0)
    x = rng.standard_normal((32, C, 64, 64), dtype=np.float32)
    w = rng.standard_normal((K, C), dtype=np.float32)
    b = rng.standard_normal((K,), dtype=np.float32)
    c = rng.random((K, C), dtype=np.float32)
    out = kernel(x=x, conv_w=w, conv_b=b, centroids=c)
    print(out.shape, out.dtype)


# revision 29
# speedup vs baseline: 1.1183x; 1.0886x over previous
"""NetVLAD Trainium2 kernel v6 — host-shipped transpose, PE-computed ssq,
flat 2D access patterns, packed constants, single-queue input staging.

Host ships x twice per image: xcp [C, P] fp16 (u-matmul stationary) and
xts [128px, 32t, 132] fp16 (vlad rhs; col 128 filled on-device with |x_p|),
plus one packed const tensor (cf f32 | cwo f16 | eb8 bf16 | cbi bf16).

Per 1024-px chunk (8 px-tiles), stages pipelined across slots:
  A: ACT sq2 = x^2 ([C,1024]); PE 8 u-matmuls (psU[128,8,64]) + 8 ssq
     matmuls (psS[:,j] = sq2_j^T @ ones); ACT lssq=ln(psS),
     invc=exp(-.5 lssq), ncol=exp(+.5 lssq) -> xts[:,:,128]
  B: DVE ls = psU*invcB (f16); DVE negm reduce; GPS d = ls+negmB;
     ACT E = exp(d + theta) -> bf16
  C: GPS EB = E*eb8; DVE scol reduce; DVE gcol=1/scol; DVE rcol;
     GPS Ep = EB[:, :, 0:56]*rcolB
  V (lag 3): PE psV[56,0:129] += Ep_j.T @ xts[:,j,0:129]
"""

import sys

for _p in ("/opt/trn_rl_repo",):
    if _p not in sys.path:
        sys.path.insert(0, _p)

import numpy as np

NIMG = 4
C = 128
K = 64
KE = 56
P = 4096
NCH = 4        # 1024-px chunks per image
TPC = 8        # 128-px tiles per chunk
NSLOT = NIMG * NCH
LAG = 3
THETA = 35.0

# packed const layout (bytes per partition):
#   [0:1028)    cf  f32 [257]: cen rows0:56 cols0:128 | theta col 128 |
#               onesrow row0 cols 129:257
#   [1028:1158) cwo f16 [65]:  w^T cols 0:64 | ones col 64
#   [1158:2182) eb8 bf16 [512]: exp(b - bmid - theta) tiled x8
#   [2182:2438) cbi bf16 [128]: identity
CPACK = 2440

_cache = {}


def _build():
    import concourse.mybir as mybir
    from concourse import bacc, tile
    from concourse.hw_specs import get_activation_tables

    f32 = mybir.dt.float32
    f16 = mybir.dt.float16
    bf16 = mybir.dt.bfloat16
    u8 = mybir.dt.uint8
    Alu = mybir.AluOpType
    Act = mybir.ActivationFunctionType

    nc = bacc.Bacc()
    xcp_in = nc.declare_dram_parameter("xcp", [NIMG, C, P], f16, isOutput=False)
    xts_in = nc.declare_dram_parameter("xts", [NIMG, C, 32, 132], f16,
                                       isOutput=False)
    cp_in = nc.declare_dram_parameter("cpack", [C, CPACK], u8, isOutput=False)
    out_ext = nc.declare_dram_parameter("out", [NIMG, KE, C], f32,
                                        isOutput=True)

    tabs = list(get_activation_tables(nc.m.arch).keys())
    tab_id = tabs.index("natural_log_exp_and_others")

    with tile.TileContext(nc) as tc:
        with (
            tc.tile_pool(name="const", bufs=1) as cpool,
            tc.tile_pool(name="xq", bufs=4) as xqpool,
            tc.tile_pool(name="xb", bufs=3) as xbpool,
            tc.tile_pool(name="xts", bufs=4) as tpool,
            tc.tile_pool(name="sq", bufs=2) as qpool,
            tc.tile_pool(name="ls", bufs=2) as lpool,
            tc.tile_pool(name="ew", bufs=3) as epool,
            tc.tile_pool(name="stats", bufs=4) as spool,
            tc.tile_pool(name="fin", bufs=2) as fpool,
            tc.tile_pool(name="psU", bufs=2, space="PSUM") as pU,
            tc.tile_pool(name="psS", bufs=2, space="PSUM") as pS,
            tc.tile_pool(name="psV", bufs=2, space="PSUM") as pV,
            tc.tile_pool(name="psT", bufs=2, space="PSUM") as pT,
        ):
            nc.scalar.add_instruction(mybir.InstLoadActFuncSet(
                name=nc.get_next_instruction_name(), ins=[], outs=[],
                act_func_set_id=tab_id))

            cpk = cpool.tile([C, CPACK], u8, tag="cpk")
            nc.sync.dma_start(cpk[:], cp_in[:])
            cf32 = cpk[:, 0:1028].bitcast(f32)
            cwo = cpk[:, 1028:1158].bitcast(f16)
            eb8 = cpk[:, 1158:2182].bitcast(bf16)
            cbi = cpk[:, 2182:2438].bitcast(bf16)

            cen = cf32[0:KE, 0:C]
            thetacol = cf32[:, 128:129]
            onesrow = cf32[0:1, 129:257]
            ident_b = cbi[:, 0:C]
            ident56_b = cbi[0:KE, 0:KE]
            wslice = cwo[:, 0:K]
            onecol = cwo[:, K:K + 1]

            # x [c,p] for u-matmul stationary; img0 in halves on sync ring,
            # xts (transposed) on gpsimd ring
            xq = []
            xts = {}
            xts[0] = tpool.tile([C, 32, 132], f16, tag="xts", name="xts0")
            for p4 in range(4):
                t = xqpool.tile([C, P // 4], f16, tag="xq", name="xqt")
                nc.sync.dma_start(t[:],
                                  xcp_in[0, :, p4 * 1024:(p4 + 1) * 1024])
                xq.append(t)
                nc.sync.dma_start(xts[0][:, p4 * 8:(p4 + 1) * 8, :],
                                  xts_in[0, :, p4 * 8:(p4 + 1) * 8, :])
            xb = {}

            def fetch_xb(img):
                xb[img] = xbpool.tile([C, P], f16, tag="xb", name="xbt")
                nc.sync.dma_start(xb[img][:], xcp_in[img])

            def fetch_xts(img):
                xts[img] = tpool.tile([C, 32, 132], f16, tag="xts",
                                      name="xtst")
                nc.sync.dma_start(xts[img][:], xts_in[img])
            fetch_xb(1)
            fetch_xts(1)

            def xchunk(c):
                img, ch = divmod(c, NCH)
                if img == 0:
                    return xq[ch][:]
                return xb[img][:, ch * 1024:(ch + 1) * 1024]

            def xsrc(c, j):
                return xchunk(c)[:, j * 128:(j + 1) * 128]

            def xv(c):
                img, ch = divmod(c, NCH)
                return xts[img][:, ch * TPC:(ch + 1) * TPC, :]

            psU = {}
            psS = {}
            st = {}
            et = {}
            ebt = {}
            ept = {}
            psV = {}
            vk = {}
            tailseq = []

            def stage_a(c):
                img, ch = divmod(c, NCH)
                if img + 1 < NIMG:
                    if ch == 0 and img + 1 not in xb:
                        fetch_xb(img + 1)
                    if ch == 1 and img + 1 not in xts:
                        fetch_xts(img + 1)
                sq2 = qpool.tile([C, 1024], f16, tag="sq2", name="sq2")
                nc.scalar.activation(sq2[:], xchunk(c), Act.Square)
                psU[c] = pU.tile([C, TPC, K], f32, tag="psU", name="psUt")
                for j in range(TPC):
                    nc.tensor.matmul(psU[c][:, j:j + 1, :], xsrc(c, j),
                                     wslice, start=True, stop=True)
                psS[c] = pS.tile([C, TPC], f32, tag="psS", name="psSt")
                for j in range(TPC):
                    nc.tensor.matmul(psS[c][:, j:j + 1],
                                     sq2[:, j * 128:(j + 1) * 128],
                                     onecol, start=True, stop=True)
                s = {}
                s["lssq"] = spool.tile([C, TPC], f32, tag="lssq", name="lssq")
                nc.scalar.activation(s["lssq"][:], psS[c][:], Act.Ln)
                s["invc"] = spool.tile([C, TPC], f32, tag="invc", name="invc")
                nc.scalar.activation(s["invc"][:], s["lssq"][:], Act.Exp,
                                     scale=-0.5)
                ncol = xv(c)[:, :, 128:129].rearrange("p t o -> p (t o)")
                nc.scalar.activation(ncol, s["lssq"][:], Act.Exp, scale=0.5)
                st[c] = s

            def stage_b(c):
                s = st[c]
                ls = lpool.tile([C, TPC * K], f16, tag="ls", name="ls")
                nc.vector.tensor_tensor(
                    ls[:].rearrange("p (t k) -> p t k", k=K), psU[c][:, :, :],
                    s["invc"][:].broadcast_to([C, TPC, K]), Alu.mult)
                s["negm"] = spool.tile([C, TPC], f32, tag="negm", name="negm")
                nc.vector.tensor_reduce(
                    s["negm"][:], ls[:].rearrange("p (t k) -> p t k", k=K),
                    axis=mybir.AxisListType.X, op=Alu.max, negate=True)
                d = lpool.tile([C, TPC * K], f16, tag="d", name="dt")
                nc.gpsimd.tensor_tensor(
                    d[:].rearrange("p (t k) -> p t k", k=K),
                    ls[:].rearrange("p (t k) -> p t k", k=K),
                    s["negm"][:].broadcast_to([C, TPC, K]), Alu.add)
                et[c] = epool.tile([C, TPC * K], bf16, tag="E", name="et")
                nc.scalar.activation(et[c][:], d[:], Act.Exp, bias=thetacol)

            def stage_c(c):
                s = st[c]
                ebt[c] = epool.tile([C, TPC * K], bf16, tag="EB", name="ebt")
                nc.gpsimd.tensor_tensor(ebt[c][:], et[c][:], eb8, Alu.mult)
                s["scol"] = spool.tile([C, TPC], f32, tag="scol", name="scol")
                nc.vector.tensor_reduce(
                    s["scol"][:], ebt[c][:].rearrange("p (t k) -> p t k", k=K),
                    axis=mybir.AxisListType.X, op=Alu.add)
                s["gcol"] = spool.tile([C, TPC], f32, tag="gcol", name="gcol")
                nc.vector.reciprocal(s["gcol"][:], s["scol"][:])
                s["rcol"] = spool.tile([C, TPC], f32, tag="rcol", name="rcol")
                nc.vector.tensor_tensor(s["rcol"][:], s["invc"][:],
                                        s["gcol"][:], Alu.mult)
                ept[c] = epool.tile([C, TPC, KE], bf16, tag="Ep", name="ept")
                nc.gpsimd.tensor_tensor(
                    ept[c][:, :, :],
                    ebt[c][:].rearrange("p (t k) -> p t k", k=K)[:, :, 0:KE],
                    s["rcol"][:].broadcast_to([C, TPC, KE]), Alu.mult)

            def vlads(c):
                img, ch = divmod(c, NCH)
                if ch == 0:
                    psV[img] = pV.tile([KE, 132], f32, tag="psV", name="psVt")
                for j in range(TPC):
                    nc.tensor.matmul(psV[img][0:KE, 0:129],
                                     ept[c][:, j, :],
                                     xv(c)[:, j, 0:129],
                                     start=(ch == 0 and j == 0),
                                     stop=(ch == NCH - 1 and j == TPC - 1))

            def tail_a(img):
                pv = psV[img]
                negs = spool.tile([KE, 1], f32, tag="negs")
                nc.vector.tensor_scalar_mul(negs[:], pv[0:KE, 128:129], -1.0)
                vk[img] = fpool.tile([KE, C], bf16, tag="vk", name="vkt")
                nc.vector.scalar_tensor_tensor(vk[img][:], cen, negs[:],
                                               pv[0:KE, 0:C],
                                               Alu.mult, Alu.add)

            def tail_b(img):
                tt = pT.tile([C, 2, 192], f32, tag="pst", name="pst")
                t1 = tt[:, 0:1, 0:KE]
                nc.tensor.matmul(t1, vk[img][:], ident56_b,
                                 start=True, stop=True)
                tr56 = spool.tile([C, KE], bf16, tag="tr56")
                ssqk = spool.tile([C, 1], f32, tag="ssqk")
                nc.scalar.activation(tr56[:], t1, Act.Square,
                                     accum_out=ssqk[:])
                ssqc = spool.tile([C, 1], f32, tag="ssqc")
                nc.vector.tensor_scalar_max(ssqc[:], ssqk[:], 1e-24)
                lk = spool.tile([C, 1], f32, tag="lk")
                nc.scalar.activation(lk[:], ssqc[:], Act.Ln)
                invk = spool.tile([C, 1], f32, tag="invk")
                nc.scalar.activation(invk[:], lk[:], Act.Exp, scale=-0.5)
                t2 = spool.tile([C, 1], f32, tag="t2")
                nc.vector.scalar_tensor_tensor(t2[:], ssqc[:], invk[:],
                                               invk[:], Alu.mult, Alu.mult)
                tot = spool.tile([1, 1], f32, tag="tot")
                nc.gpsimd.tensor_reduce(tot[:], t2[:],
                                        axis=mybir.AxisListType.C, op=Alu.add)
                totc = spool.tile([1, 1], f32, tag="totc")
                nc.vector.tensor_scalar_max(totc[:], tot[:], 1e-24)
                ltot = spool.tile([1, 1], f32, tag="ltot")
                nc.scalar.activation(ltot[:], totc[:], Act.Ln)
                fv = spool.tile([1, 1], f32, tag="fv")
                nc.scalar.activation(fv[:], ltot[:], Act.Exp, scale=-0.5)
                nc.tensor.matmul(tt[:, 1:2, 188:189], onesrow, fv[:],
                                 start=True, stop=True)
                comb = spool.tile([C, 1], f32, tag="comb")
                nc.vector.tensor_tensor(comb[:], invk[:],
                                        tt[:, 1:2, 188:189], Alu.mult)
                vnT = fpool.tile([C, KE], bf16, tag="vnT", name="vnT")
                nc.vector.tensor_scalar(vnT[:], t1, comb[:], None, Alu.mult)
                return tt, vnT

            def tail_c(img, tt, vnT):
                nc.tensor.matmul(tt[0:KE, 1:2, 0:C], vnT[:], ident_b,
                                 start=True, stop=True)
                ob = fpool.tile([KE, C], f32, tag="ob", name="ob")
                nc.scalar.activation(ob[:], tt[0:KE, 1:2, 0:C], Act.Copy)
                nc.sync.dma_start(out_ext[img], ob[:])

            for sl in range(NSLOT + LAG + 2):
                while tailseq and tailseq[0][0] <= sl:
                    tailseq.pop(0)[1]()
                if sl < NSLOT:
                    stage_a(sl)
                v = sl - LAG
                if 0 <= v < NSLOT:
                    vlads(v)
                    img, ch = divmod(v, NCH)
                    if ch == NCH - 1:
                        tail_a(img)

                        def _mk(i):
                            def _b():
                                tt, vnT = tail_b(i)
                                tailseq.append(
                                    (sl + 3, lambda: tail_c(i, tt, vnT)))
                            return _b
                        tailseq.append((sl + 1, _mk(img)))
                if 0 <= sl - 1 < NSLOT:
                    stage_b(sl - 1)
                if 0 <= sl - 2 < NSLOT:
                    stage_c(sl - 2)
            while tailseq:
                tailseq.pop(0)[1]()

    nc.compile()
    return nc


def _get_nc():
    if "nc" not in _cache:
        _cache["nc"] = _build()
    return _cache["nc"]


def _make_in_maps(x, conv_w, conv_b, centroids):
    import ml_dtypes

    x = np.asarray(x, dtype=np.float32)
    conv_w = np.asarray(conv_w, dtype=np.float32)
    conv_b = np.asarray(conv_b, dtype=np.float32)
    centroids = np.asarray(centroids, dtype=np.float32)

    N = x.shape[0]
    n_cores = 8
    per = N // n_cores
    assert per == NIMG

    xr = x.reshape(N, C, P).astype(np.float16)
    bmid = (conv_b.max() + conv_b.min()) / 2.0
    eb = np.exp((conv_b - bmid - THETA).astype(np.float64)).astype(np.float32)

    cf = np.zeros((C, 257), dtype=np.float32)
    cf[0:KE, 0:C] = centroids[:KE]
    cf[:, 128] = THETA
    cf[0, 129:257] = 1.0
    cwo = np.zeros((C, K + 1), dtype=np.float16)
    cwo[:, 0:K] = conv_w.T.astype(np.float16)
    cwo[:, K] = 1.0
    eb8 = np.broadcast_to(np.tile(eb, TPC)[None, :], (C, TPC * K)).astype(
        ml_dtypes.bfloat16)
    cbi = np.eye(C, dtype=np.float32).astype(ml_dtypes.bfloat16)

    cpack = np.concatenate([
        np.ascontiguousarray(cf).view(np.uint8),
        np.ascontiguousarray(cwo).view(np.uint8),
        np.ascontiguousarray(eb8).view(np.uint8),
        np.ascontiguousarray(cbi).view(np.uint8),
        np.zeros((C, 2), dtype=np.uint8),
    ], axis=1)
    assert cpack.shape == (C, CPACK), cpack.shape

    in_maps = []
    for i in range(n_cores):
        xc = np.ascontiguousarray(xr[i * per:(i + 1) * per])
        # xts[img, q, t, c] = x[img, c, t*128+q], padded to 132 cols
        xt = np.zeros((NIMG, C, 32, 132), dtype=np.float16)
        xt[:, :, :, 0:C] = xc.reshape(NIMG, C, 32, C).transpose(0, 3, 2, 1)
        in_maps.append({
            "xcp": xc,
            "xts": np.ascontiguousarray(xt),
            "cpack": cpack,
        })
    return in_maps


def kernel(x, conv_w, conv_b, centroids):
    from concourse.bass_utils import run_bass_kernel_spmd

    in_maps = _make_in_maps(x, conv_w, conv_b, centroids)
    nc = _get_nc()
    res = run_bass_kernel_spmd(nc, in_maps, list(range(8)))
    outs = [np.asarray(r["out"]).reshape(NIMG, KE * C) for r in res.results]
    return np.concatenate(outs, axis=0)


if __name__ == "__main__":
    rng = np.random.default_rng(0)
    x = rng.standard_normal((32, C, 64, 64), dtype=np.float32)
    w = rng.standard_normal((K, C), dtype=np.float32)
    b = rng.standard_normal((K,), dtype=np.float32)
    c = rng.random((K, C), dtype=np.float32)
    out = kernel(x=x, conv_w=w, conv_b=b, centroids=c)
    print(out.shape, out.dtype)


# revision 30
# speedup vs baseline: 1.1378x; 1.0174x over previous
"""NetVLAD Trainium2 kernel v6 — host-shipped transpose, PE-computed ssq,
flat 2D access patterns, packed constants, single-queue input staging.

Host ships x twice per image: xcp [C, P] fp16 (u-matmul stationary) and
xts [128px, 32t, 132] fp16 (vlad rhs; col 128 filled on-device with |x_p|),
plus one packed const tensor (cf f32 | cwo f16 | eb8 bf16 | cbi bf16).

Per 1024-px chunk (8 px-tiles), stages pipelined across slots:
  A: ACT sq2 = x^2 ([C,1024]); PE 8 u-matmuls (psU[128,8,64]) + 8 ssq
     matmuls (psS[:,j] = sq2_j^T @ ones); ACT lssq=ln(psS),
     invc=exp(-.5 lssq), ncol=exp(+.5 lssq) -> xts[:,:,128]
  B: DVE ls = psU*invcB (f16); DVE negm reduce; GPS d = ls+negmB;
     ACT E = exp(d + theta) -> bf16
  C: GPS EB = E*eb8; DVE scol reduce; DVE gcol=1/scol; DVE rcol;
     GPS Ep = EB[:, :, 0:56]*rcolB
  V (lag 3): PE psV[56,0:129] += Ep_j.T @ xts[:,j,0:129]
"""

import sys

for _p in ("/opt/trn_rl_repo",):
    if _p not in sys.path:
        sys.path.insert(0, _p)

import numpy as np

NIMG = 4
C = 128
K = 64
KE = 56
P = 4096
NCH = 4        # 1024-px chunks per image
TPC = 8        # 128-px tiles per chunk
NSLOT = NIMG * NCH
LAG = 3
THETA = 35.0

# packed const layout (bytes per partition):
#   [0:1028)    cf  f32 [257]: cen rows0:56 cols0:128 | theta col 128 |
#               onesrow row0 cols 129:257
#   [1028:1158) cwo f16 [65]:  w^T cols 0:64 | ones col 64
#   [1158:2182) eb8 bf16 [512]: exp(b - bmid - theta) tiled x8
#   [2182:2438) cbi bf16 [128]: identity
CPACK = 2440

_cache = {}


def _build():
    import concourse.mybir as mybir
    from concourse import bacc, tile
    from concourse.hw_specs import get_activation_tables

    f32 = mybir.dt.float32
    f16 = mybir.dt.float16
    bf16 = mybir.dt.bfloat16
    u8 = mybir.dt.uint8
    Alu = mybir.AluOpType
    Act = mybir.ActivationFunctionType

    nc = bacc.Bacc()
    xcp_in = nc.declare_dram_parameter("xcp", [NIMG, C, P], f16, isOutput=False)
    xts_in = nc.declare_dram_parameter("xts", [NIMG, C, 32, 132], f16,
                                       isOutput=False)
    cp_in = nc.declare_dram_parameter("cpack", [C, CPACK], u8, isOutput=False)
    out_ext = nc.declare_dram_parameter("out", [NIMG, KE, C], f32,
                                        isOutput=True)

    tabs = list(get_activation_tables(nc.m.arch).keys())
    tab_id = tabs.index("natural_log_exp_and_others")

    with tile.TileContext(nc) as tc:
        with (
            tc.tile_pool(name="const", bufs=1) as cpool,
            tc.tile_pool(name="xq", bufs=4) as xqpool,
            tc.tile_pool(name="xb", bufs=3) as xbpool,
            tc.tile_pool(name="xts", bufs=4) as tpool,
            tc.tile_pool(name="sq", bufs=3) as qpool,
            tc.tile_pool(name="ls", bufs=3) as lpool,
            tc.tile_pool(name="ew", bufs=4) as epool,
            tc.tile_pool(name="stats", bufs=6) as spool,
            tc.tile_pool(name="fin", bufs=2) as fpool,
            tc.tile_pool(name="psU", bufs=2, space="PSUM") as pU,
            tc.tile_pool(name="psS", bufs=2, space="PSUM") as pS,
            tc.tile_pool(name="psV", bufs=2, space="PSUM") as pV,
            tc.tile_pool(name="psT", bufs=2, space="PSUM") as pT,
        ):
            nc.scalar.add_instruction(mybir.InstLoadActFuncSet(
                name=nc.get_next_instruction_name(), ins=[], outs=[],
                act_func_set_id=tab_id))

            cpk = cpool.tile([C, CPACK], u8, tag="cpk")
            nc.sync.dma_start(cpk[:], cp_in[:])
            cf32 = cpk[:, 0:1028].bitcast(f32)
            cwo = cpk[:, 1028:1158].bitcast(f16)
            eb8 = cpk[:, 1158:2182].bitcast(bf16)
            cbi = cpk[:, 2182:2438].bitcast(bf16)

            cen = cf32[0:KE, 0:C]
            thetacol = cf32[:, 128:129]
            onesrow = cf32[0:1, 129:257]
            ident_b = cbi[:, 0:C]
            ident56_b = cbi[0:KE, 0:KE]
            wslice = cwo[:, 0:K]
            onecol = cwo[:, K:K + 1]

            # x [c,p] for u-matmul stationary; img0 in halves on sync ring,
            # xts (transposed) on gpsimd ring
            xq = []
            xts = {}
            xts[0] = tpool.tile([C, 32, 132], f16, tag="xts", name="xts0")
            for p4 in range(4):
                t = xqpool.tile([C, P // 4], f16, tag="xq", name="xqt")
                nc.sync.dma_start(t[:],
                                  xcp_in[0, :, p4 * 1024:(p4 + 1) * 1024])
                xq.append(t)
                nc.sync.dma_start(xts[0][:, p4 * 8:(p4 + 1) * 8, :],
                                  xts_in[0, :, p4 * 8:(p4 + 1) * 8, :])
            xb = {}

            def fetch_xb(img):
                xb[img] = xbpool.tile([C, P], f16, tag="xb", name="xbt")
                nc.sync.dma_start(xb[img][:], xcp_in[img])

            def fetch_xts(img):
                xts[img] = tpool.tile([C, 32, 132], f16, tag="xts",
                                      name="xtst")
                nc.sync.dma_start(xts[img][:], xts_in[img])
            fetch_xb(1)
            fetch_xts(1)

            def xchunk(c):
                img, ch = divmod(c, NCH)
                if img == 0:
                    return xq[ch][:]
                return xb[img][:, ch * 1024:(ch + 1) * 1024]

            def xsrc(c, j):
                return xchunk(c)[:, j * 128:(j + 1) * 128]

            def xv(c):
                img, ch = divmod(c, NCH)
                return xts[img][:, ch * TPC:(ch + 1) * TPC, :]

            psU = {}
            psS = {}
            st = {}
            et = {}
            ebt = {}
            ept = {}
            psV = {}
            vk = {}
            tailseq = []

            def stage_a(c):
                img, ch = divmod(c, NCH)
                if img + 1 < NIMG:
                    if ch == 0 and img + 1 not in xb:
                        fetch_xb(img + 1)
                    if ch == 1 and img + 1 not in xts:
                        fetch_xts(img + 1)
                sq2 = qpool.tile([C, 1024], f16, tag="sq2", name="sq2")
                nc.scalar.activation(sq2[:], xchunk(c), Act.Square)
                psU[c] = pU.tile([C, TPC, K], f32, tag="psU", name="psUt")
                for j in range(TPC):
                    nc.tensor.matmul(psU[c][:, j:j + 1, :], xsrc(c, j),
                                     wslice, start=True, stop=True)
                psS[c] = pS.tile([C, TPC], f32, tag="psS", name="psSt")
                for j in range(TPC):
                    nc.tensor.matmul(psS[c][:, j:j + 1],
                                     sq2[:, j * 128:(j + 1) * 128],
                                     onecol, start=True, stop=True)
                s = {}
                s["lssq"] = spool.tile([C, TPC], f32, tag="lssq", name="lssq")
                nc.scalar.activation(s["lssq"][:], psS[c][:], Act.Ln)
                s["invc"] = spool.tile([C, TPC], f32, tag="invc", name="invc")
                nc.scalar.activation(s["invc"][:], s["lssq"][:], Act.Exp,
                                     scale=-0.5)
                ncol = xv(c)[:, :, 128:129].rearrange("p t o -> p (t o)")
                nc.scalar.activation(ncol, s["lssq"][:], Act.Exp, scale=0.5)
                st[c] = s

            def stage_b(c):
                s = st[c]
                ls = lpool.tile([C, TPC * K], f16, tag="ls", name="ls")
                nc.vector.tensor_tensor(
                    ls[:].rearrange("p (t k) -> p t k", k=K), psU[c][:, :, :],
                    s["invc"][:].broadcast_to([C, TPC, K]), Alu.mult)
                s["negm"] = spool.tile([C, TPC], f32, tag="negm", name="negm")
                nc.vector.tensor_reduce(
                    s["negm"][:], ls[:].rearrange("p (t k) -> p t k", k=K),
                    axis=mybir.AxisListType.X, op=Alu.max, negate=True)
                d = lpool.tile([C, TPC * K], f16, tag="d", name="dt")
                nc.gpsimd.tensor_tensor(
                    d[:].rearrange("p (t k) -> p t k", k=K),
                    ls[:].rearrange("p (t k) -> p t k", k=K),
                    s["negm"][:].broadcast_to([C, TPC, K]), Alu.add)
                et[c] = epool.tile([C, TPC * K], bf16, tag="E", name="et")
                nc.scalar.activation(et[c][:], d[:], Act.Exp, bias=thetacol)

            def stage_c(c):
                s = st[c]
                ebt[c] = epool.tile([C, TPC * K], bf16, tag="EB", name="ebt")
                nc.gpsimd.tensor_tensor(ebt[c][:], et[c][:], eb8, Alu.mult)
                s["scol"] = spool.tile([C, TPC], f32, tag="scol", name="scol")
                nc.vector.tensor_reduce(
                    s["scol"][:], ebt[c][:].rearrange("p (t k) -> p t k", k=K),
                    axis=mybir.AxisListType.X, op=Alu.add)
                s["gcol"] = spool.tile([C, TPC], f32, tag="gcol", name="gcol")
                nc.vector.reciprocal(s["gcol"][:], s["scol"][:])
                s["rcol"] = spool.tile([C, TPC], f32, tag="rcol", name="rcol")
                nc.vector.tensor_tensor(s["rcol"][:], s["invc"][:],
                                        s["gcol"][:], Alu.mult)
                ept[c] = epool.tile([C, TPC, KE], bf16, tag="Ep", name="ept")
                nc.gpsimd.tensor_tensor(
                    ept[c][:, :, :],
                    ebt[c][:].rearrange("p (t k) -> p t k", k=K)[:, :, 0:KE],
                    s["rcol"][:].broadcast_to([C, TPC, KE]), Alu.mult)

            def vlads(c):
                img, ch = divmod(c, NCH)
                if ch == 0:
                    psV[img] = pV.tile([KE, 132], f32, tag="psV", name="psVt")
                for j in range(TPC):
                    nc.tensor.matmul(psV[img][0:KE, 0:129],
                                     ept[c][:, j, :],
                                     xv(c)[:, j, 0:129],
                                     start=(ch == 0 and j == 0),
                                     stop=(ch == NCH - 1 and j == TPC - 1))

            def tail_a(img):
                pv = psV[img]
                negs = spool.tile([KE, 1], f32, tag="negs")
                nc.vector.tensor_scalar_mul(negs[:], pv[0:KE, 128:129], -1.0)
                vk[img] = fpool.tile([KE, C], bf16, tag="vk", name="vkt")
                nc.vector.scalar_tensor_tensor(vk[img][:], cen, negs[:],
                                               pv[0:KE, 0:C],
                                               Alu.mult, Alu.add)

            def tail_b(img):
                tt = pT.tile([C, 2, 192], f32, tag="pst", name="pst")
                t1 = tt[:, 0:1, 0:KE]
                nc.tensor.matmul(t1, vk[img][:], ident56_b,
                                 start=True, stop=True)
                tr56 = spool.tile([C, KE], bf16, tag="tr56")
                ssqk = spool.tile([C, 1], f32, tag="ssqk")
                nc.scalar.activation(tr56[:], t1, Act.Square,
                                     accum_out=ssqk[:])
                ssqc = spool.tile([C, 1], f32, tag="ssqc")
                nc.vector.tensor_scalar_max(ssqc[:], ssqk[:], 1e-24)
                lk = spool.tile([C, 1], f32, tag="lk")
                nc.scalar.activation(lk[:], ssqc[:], Act.Ln)
                invk = spool.tile([C, 1], f32, tag="invk")
                nc.scalar.activation(invk[:], lk[:], Act.Exp, scale=-0.5)
                t2 = spool.tile([C, 1], f32, tag="t2")
                nc.vector.scalar_tensor_tensor(t2[:], ssqc[:], invk[:],
                                               invk[:], Alu.mult, Alu.mult)
                tot = spool.tile([1, 1], f32, tag="tot")
                nc.gpsimd.tensor_reduce(tot[:], t2[:],
                                        axis=mybir.AxisListType.C, op=Alu.add)
                totc = spool.tile([1, 1], f32, tag="totc")
                nc.vector.tensor_scalar_max(totc[:], tot[:], 1e-24)
                ltot = spool.tile([1, 1], f32, tag="ltot")
                nc.scalar.activation(ltot[:], totc[:], Act.Ln)
                fv = spool.tile([1, 1], f32, tag="fv")
                nc.scalar.activation(fv[:], ltot[:], Act.Exp, scale=-0.5)
                nc.tensor.matmul(tt[:, 1:2, 188:189], onesrow, fv[:],
                                 start=True, stop=True)
                comb = spool.tile([C, 1], f32, tag="comb")
                nc.vector.tensor_tensor(comb[:], invk[:],
                                        tt[:, 1:2, 188:189], Alu.mult)
                vnT = fpool.tile([C, KE], bf16, tag="vnT", name="vnT")
                nc.vector.tensor_scalar(vnT[:], t1, comb[:], None, Alu.mult)
                return tt, vnT

            def tail_c(img, tt, vnT):
                nc.tensor.matmul(tt[0:KE, 1:2, 0:C], vnT[:], ident_b,
                                 start=True, stop=True)
                ob = fpool.tile([KE, C], f32, tag="ob", name="ob")
                nc.scalar.activation(ob[:], tt[0:KE, 1:2, 0:C], Act.Copy)
                nc.sync.dma_start(out_ext[img], ob[:])

            for sl in range(NSLOT + LAG + 2):
                while tailseq and tailseq[0][0] <= sl:
                    tailseq.pop(0)[1]()
                if sl < NSLOT:
                    stage_a(sl)
                v = sl - LAG
                if 0 <= v < NSLOT:
                    vlads(v)
                    img, ch = divmod(v, NCH)
                    if ch == NCH - 1:
                        tail_a(img)

                        def _mk(i):
                            def _b():
                                tt, vnT = tail_b(i)
                                tailseq.append(
                                    (sl + 3, lambda: tail_c(i, tt, vnT)))
                            return _b
                        tailseq.append((sl + 1, _mk(img)))
                if 0 <= sl - 1 < NSLOT:
                    stage_b(sl - 1)
                if 0 <= sl - 2 < NSLOT:
                    stage_c(sl - 2)
            while tailseq:
                tailseq.pop(0)[1]()

    nc.compile()
    return nc


def _get_nc():
    if "nc" not in _cache:
        _cache["nc"] = _build()
    return _cache["nc"]


def _make_in_maps(x, conv_w, conv_b, centroids):
    import ml_dtypes

    x = np.asarray(x, dtype=np.float32)
    conv_w = np.asarray(conv_w, dtype=np.float32)
    conv_b = np.asarray(conv_b, dtype=np.float32)
    centroids = np.asarray(centroids, dtype=np.float32)

    N = x.shape[0]
    n_cores = 8
    per = N // n_cores
    assert per == NIMG

    xr = x.reshape(N, C, P).astype(np.float16)
    bmid = (conv_b.max() + conv_b.min()) / 2.0
    eb = np.exp((conv_b - bmid - THETA).astype(np.float64)).astype(np.float32)

    cf = np.zeros((C, 257), dtype=np.float32)
    cf[0:KE, 0:C] = centroids[:KE]
    cf[:, 128] = THETA
    cf[0, 129:257] = 1.0
    cwo = np.zeros((C, K + 1), dtype=np.float16)
    cwo[:, 0:K] = conv_w.T.astype(np.float16)
    cwo[:, K] = 1.0
    eb8 = np.broadcast_to(np.tile(eb, TPC)[None, :], (C, TPC * K)).astype(
        ml_dtypes.bfloat16)
    cbi = np.eye(C, dtype=np.float32).astype(ml_dtypes.bfloat16)

    cpack = np.concatenate([
        np.ascontiguousarray(cf).view(np.uint8),
        np.ascontiguousarray(cwo).view(np.uint8),
        np.ascontiguousarray(eb8).view(np.uint8),
        np.ascontiguousarray(cbi).view(np.uint8),
        np.zeros((C, 2), dtype=np.uint8),
    ], axis=1)
    assert cpack.shape == (C, CPACK), cpack.shape

    in_maps = []
    for i in range(n_cores):
        xc = np.ascontiguousarray(xr[i * per:(i + 1) * per])
        # xts[img, q, t, c] = x[img, c, t*128+q], padded to 132 cols
        xt = np.zeros((NIMG, C, 32, 132), dtype=np.float16)
        xt[:, :, :, 0:C] = xc.reshape(NIMG, C, 32, C).transpose(0, 3, 2, 1)
        in_maps.append({
            "xcp": xc,
            "xts": np.ascontiguousarray(xt),
            "cpack": cpack,
        })
    return in_maps


def kernel(x, conv_w, conv_b, centroids):
    from concourse.bass_utils import run_bass_kernel_spmd

    in_maps = _make_in_maps(x, conv_w, conv_b, centroids)
    nc = _get_nc()
    res = run_bass_kernel_spmd(nc, in_maps, list(range(8)))
    outs = [np.asarray(r["out"]).reshape(NIMG, KE * C) for r in res.results]
    return np.concatenate(outs, axis=0)


if __name__ == "__main__":
    rng = np.random.default_rng(0)
    x = rng.standard_normal((32, C, 64, 64), dtype=np.float32)
    w = rng.standard_normal((K, C), dtype=np.float32)
    b = rng.standard_normal((K,), dtype=np.float32)
    c = rng.random((K, C), dtype=np.float32)
    out = kernel(x=x, conv_w=w, conv_b=b, centroids=c)
    print(out.shape, out.dtype)
